# revision 5
# baseline (speedup 1.0000x reference)
"""Expert-parallel MoE FFN kernel for 8 Trainium2 NeuronCores.

Math (per expert e): out = gelu(x_e @ w1_e + b1_e) @ w2_e + b2_e
  x: [B=2, E=8, N=1024, D=1024], w1: [E, D, F=4096], b1: [E, F],
  w2: [E, F, D], b2: [E, D]  ->  out: [B, E, N, D]

Sharding: one expert per core (the e axis of every tensor), outputs
gathered on host — the distributed path the original module implements
with all_gather + split_by_rank.

Per-core kernel (all matmuls bf16 into fp32 PSUM, measured at the PE
clock floor for this part — the moving operand streams 1 col/cycle and
weight loads are fully hidden, so 2048 N=512 matmuls is the hardware
minimum):
  - x arrives host-transposed as xT [d, tok] (bf16), so the device does
    no transposes at all. x and w2 are SBUF-resident the whole kernel;
    w1 streams per 512-wide f-group.
  - mm1: psum[f128, tok512] += w1[d, f].T @ xT[d, tok] over 8 k-tiles;
    ACT applies exact GELU with per-partition bias b1 while moving
    PSUM -> SBUF hT [f, tok] (bf16). Tokens processed in halves of 1024
    so hT fits SBUF alongside resident w2.
  - mm2: psum[tok128, d512] += hT[f, tok].T @ w2[f, d] over 32 f-tiles;
    DVE fuses the b2 broadcast add while moving PSUM -> SBUF fp32, then
    the result DMAs out in natural [tok, d] layout.
"""

import sys

for _p in ("/opt/trn_rl_repo", "/opt/pypackages"):
    if _p not in sys.path:
        sys.path.append(_p)

import numpy as np

B, E, N, D, F = 2, 8, 1024, 1024, 4096
TOK = B * N          # tokens per expert (2048)
HALF = TOK // 2      # token half (1024)
nD = D // 128        # 8 d-tiles
nF = F // 128        # 32 f-tiles
TB = 512             # matmul moving width

_CACHE: dict = {}


def _build(reps: int = 1):
    import concourse.bacc as bacc
    import concourse.bass as bass
    import concourse.tile as tile
    from concourse import mybir

    F32 = mybir.dt.float32
    BF16 = mybir.dt.bfloat16
    GELU = mybir.ActivationFunctionType.Gelu
    MULT = mybir.AluOpType.mult
    ADD = mybir.AluOpType.add

    nc = bacc.Bacc("TRN2", target_bir_lowering=False, debug=False, num_devices=8)

    xh = nc.dram_tensor("xh", [D, TOK], BF16, kind="ExternalInput").ap()
    w1h = nc.dram_tensor("w1h", [nD, 128, F], BF16, kind="ExternalInput").ap()
    w2h = nc.dram_tensor("w2h", [nF, 128, D], BF16, kind="ExternalInput").ap()
    b1 = nc.dram_tensor("b1", [F], F32, kind="ExternalInput").ap()
    b2 = nc.dram_tensor("b2", [D], F32, kind="ExternalInput").ap()
    out = nc.dram_tensor("out", [TOK, D], F32, kind="ExternalOutput").ap()

    x_v = xh.rearrange("(dt p) tok -> p dt tok", p=128)
    w1_vg = w1h.rearrange("dt p (fg f) -> p dt fg f", f=512)
    w2_v = w2h.rearrange("ft p d -> p ft d")
    out_v = out.rearrange("(hf tt p) (dh c) -> hf tt p dh c", hf=2, p=128, c=TB)

    with tile.TileContext(nc) as tc:
        with (
            tc.tile_pool(name="consts", bufs=1) as consts,
            tc.tile_pool(name="xp", bufs=1) as xp,
            tc.tile_pool(name="w2p", bufs=1) as w2p,
            tc.tile_pool(name="hp", bufs=1) as hp,
            tc.tile_pool(name="w1p", bufs=2) as w1p,
            tc.tile_pool(name="op", bufs=4) as op,
            tc.tile_pool(name="ps", bufs=8, space="PSUM") as ps,
        ):
            b1_t = consts.tile([128, nF], F32, tag="b1")
            nc.sync.dma_start(out=b1_t, in_=b1.rearrange("(fc p) -> p fc", p=128))
            b2_t = consts.tile([128, D], F32, tag="b2")
            nc.gpsimd.dma_start(
                out=b2_t,
                in_=bass.AP(tensor=b2.tensor, offset=b2.offset, ap=[[0, 128], [1, D]]),
            )

            # x resident as 4 chunk tiles; only chunk 0 loads up front so
            # mm1 starts after ~2MB of sync-ring traffic. x1-x3 are issued
            # behind the first w1 groups; w2 rides the DVE ring in parallel.
            xq_t = []
            for q in range(TOK // TB):
                xq = xp.tile([128, nD, TB], BF16, tag=f"xq{q}")
                if q == 0:
                    nc.sync.dma_start(out=xq, in_=x_v[:, :, 0:TB])
                xq_t.append(xq)
            w2h_t = w2p.tile([128, nF, D], BF16, tag="w2h")
            nc.scalar.dma_start(out=w2h_t, in_=w2_v)

            for rep in range(reps):
                for hf in range(2):  # token halves
                    t0 = hf * HALF
                    # ---- mm1 + gelu -> hT [f, tok] (bf16) ----
                    hh_t = hp.tile(
                        [128, nF, HALF], BF16, tag="hh", name=f"hh_{rep}_{hf}"
                    )
                    for fg in range(nF // 4):  # f-groups of 512
                        w1h_g = w1p.tile(
                            [128, nD, 512], BF16, tag="w1h",
                            name=f"w1h_{rep}_{hf}_{fg}",
                        )
                        nc.sync.dma_start(out=w1h_g, in_=w1_vg[:, :, fg])
                        if rep == 0 and hf == 0 and 1 + fg < len(xq_t):
                            q = 1 + fg
                            nc.sync.dma_start(
                                out=xq_t[q], in_=x_v[:, :, q * TB : (q + 1) * TB]
                            )
                        for fi in range(4):
                            fc = fg * 4 + fi
                            fs = fi * 128
                            for thc in range(HALF // TB):
                                xq = xq_t[(t0 + thc * TB) // TB]
                                pt = ps.tile(
                                    [128, TB], F32, tag="ps",
                                    name=f"p1_{rep}_{hf}_{fc}_{thc}",
                                )
                                for k in range(nD):
                                    nc.tensor.matmul(
                                        pt,
                                        w1h_g[:, k, fs : fs + 128],
                                        xq[:, k, :],
                                        start=(k == 0),
                                        stop=(k == nD - 1),
                                    )
                                nc.scalar.activation(
                                    hh_t[:, fc, thc * TB : thc * TB + TB], pt,
                                    GELU, bias=b1_t[:, fc : fc + 1], scale=1.0,
                                )

                    # ---- mm2 + b2 -> out [tok, d] (fp32) ----
                    for dh in range(D // TB):
                        ds = dh * TB
                        for tt in range(8):  # token tiles of 128 in this half
                            hts = tt * 128
                            pt = ps.tile(
                                [128, TB], F32, tag="ps",
                                name=f"p2_{rep}_{hf}_{dh}_{tt}",
                            )
                            for k in range(nF):
                                nc.tensor.matmul(
                                    pt,
                                    hh_t[:, k, hts : hts + 128],
                                    w2h_t[:, k, ds : ds + TB],
                                    start=(k == 0),
                                    stop=(k == nF - 1),
                                )
                            ot = op.tile(
                                [128, TB], F32, tag="o",
                                name=f"o_{rep}_{hf}_{dh}_{tt}",
                            )
                            nc.vector.scalar_tensor_tensor(
                                out=ot,
                                in0=pt,
                                scalar=1.0,
                                in1=b2_t[:, ds : ds + TB],
                                op0=MULT,
                                op1=ADD,
                            )
                            nc.scalar.dma_start(out=out_v[hf, tt, :, dh], in_=ot)

    nc.compile()
    return nc


def _get_nc(reps: int = 1):
    key = f"nc{reps}"
    if key not in _CACHE:
        _CACHE[key] = _build(reps)
    return _CACHE[key]


def _prep_core_inputs(x, w1, b1, w2, b2, e):
    from concourse import mybir

    BFnp = mybir.dt.np(mybir.dt.bfloat16)
    xT = np.ascontiguousarray(x[:, e].reshape(TOK, D).T)  # [D, TOK]
    return {
        "xh": xT.astype(BFnp),
        "w1h": np.ascontiguousarray(w1[e].astype(BFnp).reshape(nD, 128, F)),
        "w2h": np.ascontiguousarray(w2[e].astype(BFnp).reshape(nF, 128, D)),
        "b1": np.ascontiguousarray(b1[e]),
        "b2": np.ascontiguousarray(b2[e]),
    }


def kernel(x, w1, b1, w2, b2):
    from concourse.bass_utils import run_bass_kernel_spmd

    x = np.asarray(x, dtype=np.float32)
    w1 = np.asarray(w1, dtype=np.float32)
    b1 = np.asarray(b1, dtype=np.float32)
    w2 = np.asarray(w2, dtype=np.float32)
    b2 = np.asarray(b2, dtype=np.float32)

    nc = _get_nc()
    in_maps = [_prep_core_inputs(x, w1, b1, w2, b2, e) for e in range(E)]
    res = run_bass_kernel_spmd(nc, in_maps, list(range(E)))
    out = np.empty((B, E, N, D), np.float32)
    for e in range(E):
        out[:, e] = res.results[e]["out"].reshape(B, N, D)
    return out
